# revision 9
# baseline (speedup 1.0000x reference)
"""Trainium2 Bass kernel for WeightedCorrelationLayer (nn_CorrNetImpl).

Math (per batch b, G=1):
  x1 = temporal shift of x (frame t pairs with frame t-1; frame 0 with itself)
  x2 = spatially zero-padded x (pad=3)
  out[b, o=(dy,dx), t, h, w] = (1/C) * sum_c w[c,t,dy,dx] * x1[b,c,t,h,w]
                                       * x2[b,c,t,h+dy,w+dx]

Strategy:
  - Data-parallel over batch: core i handles batch i (B=8, 8 cores).
  - Per core, process t in pairs (t0,t1): SBUF tiles hold both frames on
    the 128 partitions as (delta, c) -> partition delta*64+c.
  - For each of the 49 offsets: DVE/ACT elementwise product
    P[k, hw] = x1[k, hw] * x2pad[k, (h+dy)*WP + (w+dx)]  (shifted AP view)
  - Channel reduction via TensorE: M=2 matmul per (offset, hw-chunk) with a
    host-precomputed block lhsT [128, 2] (weights/C baked in, zeros in the
    off-diagonal half), accumulated nowhere -- each offset writes its own
    PSUM partition pair (2o, 2o+1).
  - PSUM -> SBUF copy, then one DMA per t-pair to the DRAM output.
"""

import numpy as np

import concourse.bacc as bacc
import concourse.mybir as mybir
import concourse.tile as tile
from concourse import bass_utils

B, C, T, H, W = 8, 64, 32, 56, 56
K = 7
PAD = (K - 1) // 2
NOFF = K * K
N_CORES = 8

F32 = mybir.dt.float32
F32R = mybir.dt.float32r


def build(C=C, T=T, H=H, W=W, K=K, chunk=448, use_f32r=True, n_cores=N_CORES,
          p_bufs=4):
    PADL = (K - 1) // 2
    HW = H * W
    WP = W + 2 * PADL
    HP = H + 2 * PADL
    NO = K * K
    NPAIR = T // 2
    assert HW % chunk == 0 and chunk <= 512
    nchunk = HW // chunk
    mmdt = F32R if use_f32r else F32

    nc = bacc.Bacc("TRN2", target_bir_lowering=False, debug=False,
                   num_devices=n_cores)
    x_d = nc.dram_tensor("x", [C, T, H, W], F32, kind="ExternalInput")
    w_d = nc.dram_tensor("wblk", [NPAIR, 128, 32 * NO], mmdt,
                         kind="ExternalInput")
    o_d = nc.dram_tensor("out", [NO, T, HW], F32, kind="ExternalOutput")

    xap = x_d.ap()
    wap = w_d.ap()
    oap = o_d.ap()

    with tile.TileContext(nc) as tc:
        with (
            tc.tile_pool(name="x2", bufs=1) as x2pool,
            tc.tile_pool(name="x1", bufs=2) as x1pool,
            tc.tile_pool(name="wt", bufs=2) as wtpool,
            tc.tile_pool(name="prod", bufs=p_bufs) as ppool,
            tc.tile_pool(name="ps", bufs=8, space="PSUM") as pspool,
            tc.tile_pool(name="ot", bufs=2) as otpool,
        ):
            x2bufs = [x2pool.tile([128, HP * WP], F32, tag=f"x2_{i}",
                                  name=f"x2_{i}")
                      for i in range(2)]
            for tl in x2bufs:
                nc.vector.memset(tl[:, :], 0.0)

            for j in range(NPAIR):
                t0 = 2 * j
                t1 = t0 + 1

                x1t = x1pool.tile([128, HW], F32)
                nc.sync.dma_start(
                    x1t[0:C, :],
                    xap[:, max(t0 - 1, 0)].rearrange("c h w -> c (h w)"))
                nc.sync.dma_start(
                    x1t[C:2 * C, :],
                    xap[:, t0].rearrange("c h w -> c (h w)"))

                x2t = x2bufs[j % 2]
                x2v = x2t[:, :].rearrange("p (h w) -> p h w", w=WP)
                nc.sync.dma_start(
                    x2v[0:C, PADL:PADL + H, PADL:PADL + W], xap[:, t0])
                nc.sync.dma_start(
                    x2v[C:2 * C, PADL:PADL + H, PADL:PADL + W], xap[:, t1])

                wtt = wtpool.tile([128, 32 * NO], mmdt)
                nc.sync.dma_start(wtt[:, :], wap[j])

                x1v = x1t[:, :].rearrange("p (h w) -> p h w", w=W)
                outt = otpool.tile([2 * NO, HW], F32)

                ps = [None] * nchunk
                for dy in range(K):
                    for dx in range(K):
                        o = dy * K + dx
                        g = o // 16
                        i = o % 16
                        first = (i == 0)
                        last = (i == 15 or o == NO - 1)
                        ngrp = min(16, NO - 16 * g)  # offsets in this group
                        pt = ppool.tile([128, HW], mmdt)
                        pv = pt[:, :].rearrange("p (h w) -> p h w", w=W)
                        nc.any.tensor_mul(pv, x1v, x2v[:, dy:dy + H, dx:dx + W])
                        m = 2 * ngrp
                        for ch in range(nchunk):
                            if first:
                                ps[ch] = pspool.tile(
                                    [32, chunk], F32,
                                    name=f"ps_{j}_{g}_{ch}", tag="ps")
                            nc.tensor.matmul(
                                ps[ch][0:m, :],
                                wtt[:, 32 * o:32 * o + m],
                                pt[:, ch * chunk:(ch + 1) * chunk],
                                start=first, stop=last)
                            if last:
                                nc.any.tensor_copy(
                                    outt[32 * g:32 * g + m,
                                         ch * chunk:(ch + 1) * chunk],
                                    ps[ch][0:m, :])

                for g in range((NO + 15) // 16):
                    ng = min(16, NO - 16 * g)
                    dst = oap[16 * g:16 * g + ng, t0:t0 + 2, :]
                    nc.sync.dma_start(
                        dst.rearrange("o d n -> d o n"),
                        outt[32 * g:32 * g + 2 * ng, :])



    nc.compile()
    return nc


def make_wblk(filter_weight, C=C, T=T, K=K):
    """Host-side repack of (C, T, K, K) weights into per-pair lhsT blocks
    [NPAIR, 128, 32*K*K]. Offset o (group g=o//16, slot i=o%16, group
    size ng) gets a [128, 2*ng] block at cols 32o..: col i rows 0..C-1
    hold w[:, 2j, o]/C (frame t0), col ng+i rows C..2C-1 hold
    w[:, 2j+1, o]/C (frame t1). Each block is the stationary operand of
    an M=2*ng matmul; PSUM rows are (delta, i) delta-major."""
    NO = K * K
    NPAIR = T // 2
    fw = np.asarray(filter_weight, np.float32).reshape(C, T, NO) / C
    wblk = np.zeros((NPAIR, 128, 32 * NO), np.float32)
    for o in range(NO):
        g, i = divmod(o, 16)
        ng = min(16, NO - 16 * g)
        wblk[:, 0:C, 32 * o + i] = fw[:, 0::2, o].T
        wblk[:, C:2 * C, 32 * o + ng + i] = fw[:, 1::2, o].T
    return wblk


_NC_CACHE = {}


def _get_nc():
    if "nc" not in _NC_CACHE:
        _NC_CACHE["nc"] = build()
    return _NC_CACHE["nc"]


def kernel(x, filter_weight):
    x = np.ascontiguousarray(np.asarray(x, np.float32))
    wblk = make_wblk(filter_weight)
    nc = _get_nc()
    in_maps = [{"x": x[b], "wblk": wblk} for b in range(B)]
    res = bass_utils.run_bass_kernel_spmd(nc, in_maps,
                                          core_ids=list(range(N_CORES)))
    out = np.stack([res.results[b]["out"] for b in range(B)])
    return out.reshape(B, NOFF, T, H, W)


# revision 17
# speedup vs baseline: 1.2308x; 1.2308x over previous
"""Trainium2 Bass kernel for WeightedCorrelationLayer (nn_CorrNetImpl).

Math (per batch b, G=1):
  x1 = temporal shift of x (frame t pairs with frame t-1; frame 0 with itself)
  x2 = spatially zero-padded x (pad=3)
  out[b, o=(dy,dx), t, h, w] = (1/C) * sum_c w[c,t,dy,dx] * x1[b,c,t,h,w]
                                       * x2[b,c,t,h+dy,w+dx]

Strategy:
  - Data-parallel over batch: core i handles batch i (B=8, 8 cores).
  - Per core, process t in pairs (t0,t1): SBUF tiles hold both frames on
    the 128 partitions as (delta, c) -> partition delta*64+c.
  - ScalarE casts the f32 frames to fp16 (even- and odd-aligned copies of
    the padded frame so every shifted view stays 4B-aligned for the DVE
    2x perf mode).
  - VectorE computes the elementwise products
    P[k, (dy, h, w)] = x1[k, hw] * x2pad[k, (h+dy)*WPE + (w+dx)]
    batching a run of dy values per instruction (broadcast x1 via a
    0-stride dim, dy via a WPE-stride dim) to amortize DVE op overhead.
  - Channel reduction on TensorE: offsets grouped by dx-pair (14 offsets,
    M=28) per PSUM tile; each offset is one fp16 matmul with a
    host-precomputed block lhsT (weights/C baked in; frame t0 columns
    then frame t1 columns), PSUM rows (delta, i) delta-major with
    i = (dx - 2g)*7 + dy, accumulating the group's offsets.
  - ScalarE copies PSUM -> SBUF (staging group g at 32-aligned row base
    32g, as compute APs need 32-aligned partition bases), then 8 DMAs per
    t-pair write DRAM.
"""

import dataclasses

import numpy as np

import concourse.bacc as bacc
import concourse.mybir as mybir
import concourse.tile as tile
from concourse import bass_utils

B, C, T, H, W = 8, 64, 32, 56, 56
K = 7
PAD = (K - 1) // 2
NOFF = K * K
N_CORES = 8

F32 = mybir.dt.float32
F32R = mybir.dt.float32r
FP16 = mybir.dt.float16
DYQ4 = ((0, 4), (4, 3))
DYQ2 = ((0, 2), (2, 2), (4, 2), (6, 1))
DYQ1 = tuple((d, 1) for d in range(7))


def _freeview(ap, dims, off):
    """Free-dim strided view of a flat [128, N] tile AP."""
    return dataclasses.replace(ap, ap=[ap.ap[0]] + dims, offset=ap.offset + off)


def build(C=C, T=T, H=H, W=W, K=K, chunk=448, mode="fp16", n_cores=N_CORES,
          p_bufs=6, reps=1, dyq=DYQ2, x2f_bufs=2):
    PADL = (K - 1) // 2
    HW = H * W
    WPE = -(-(W + 2 * PADL) // 2) * 2  # even padded width for alignment
    HP = H + 2 * PADL
    NO = K * K
    NPAIR = T // 2
    NG = (K + 1) // 2  # dx-pair groups
    assert HW % chunk == 0 and chunk <= 512
    nchunk = HW // chunk
    assert mode in ("fp16", "f32r")
    lowp = mode == "fp16"
    mmdt = FP16 if lowp else F32R

    def grp(dy, dx):
        g = dx // 2
        ng = K * min(2, K - 2 * g)  # offsets in group
        i = (dx - 2 * g) * K + dy
        return g, i, ng

    nc = bacc.Bacc("TRN2", target_bir_lowering=False, debug=False,
                   num_devices=n_cores)
    x_d = nc.dram_tensor("x", [C, T, H, W], F32, kind="ExternalInput")
    w_d = nc.dram_tensor("wblk", [NPAIR, 128, 32 * NO], mmdt,
                         kind="ExternalInput")
    o_d = nc.dram_tensor("out", [NO, T, HW], F32, kind="ExternalOutput")

    xap = x_d.ap()
    wap = w_d.ap()
    oap = o_d.ap()

    with tile.TileContext(nc) as tc:
        with (
            tc.tile_pool(name="x2", bufs=1) as x2pool,
            tc.tile_pool(name="x1", bufs=2) as x1pool,
            tc.tile_pool(name="wt", bufs=2) as wtpool,
            tc.tile_pool(name="prod", bufs=p_bufs) as ppool,
            tc.tile_pool(name="ps", bufs=8, space="PSUM") as pspool,
            tc.tile_pool(name="ot", bufs=2) as otpool,
        ):
            NPL = HP * WPE  # padded plane size
            x2f = [x2pool.tile([128, NPL], F32, tag=f"x2f_{i}",
                               name=f"x2f_{i}") for i in range(x2f_bufs)]
            for tl in x2f:
                nc.gpsimd.memset(tl[:, :], 0.0)
            if lowp:
                x2e = [x2pool.tile([128, NPL], FP16, tag=f"x2e_{i}",
                                   name=f"x2e_{i}") for i in range(2)]
                x2o = [x2pool.tile([128, NPL], FP16, tag=f"x2o_{i}",
                                   name=f"x2o_{i}") for i in range(2)]
                for tl in x2o:
                    nc.gpsimd.memset(tl[:, :], 0.0)

            for j in range(NPAIR * reps):
                j = j % NPAIR
                t0 = 2 * j
                t1 = t0 + 1

                x1f = x1pool.tile([128, HW], F32, name="x1f")
                nc.sync.dma_start(
                    x1f[0:C, :],
                    xap[:, max(t0 - 1, 0)].rearrange("c h w -> c (h w)"))
                nc.sync.dma_start(
                    x1f[C:2 * C, :],
                    xap[:, t0].rearrange("c h w -> c (h w)"))

                x2t = x2f[j % x2f_bufs]
                x2fv = x2t[:, :].rearrange("p (h w) -> p h w", w=WPE)
                nc.sync.dma_start(
                    x2fv[0:C, PADL:PADL + H, PADL:PADL + W], xap[:, t0])
                nc.sync.dma_start(
                    x2fv[C:2 * C, PADL:PADL + H, PADL:PADL + W], xap[:, t1])

                if lowp:
                    x1t = x1pool.tile([128, HW], FP16, name="x1t")
                    nc.scalar.copy(x1t[:, :], x1f[:, :])
                    x2et = x2e[j % 2]
                    x2ot = x2o[j % 2]
                    nc.scalar.copy(x2et[:, :], x2t[:, :])
                    nc.scalar.copy(x2ot[:, 0:NPL - 1], x2t[:, 1:NPL])
                else:
                    x1t = x1f
                    x2et = x2t
                    x2ot = None

                wtt = wtpool.tile([128, 32 * NO], mmdt, name="wtt")
                nc.sync.dma_start(wtt[:, :], wap[j])

                outt = otpool.tile([32 * NG - 32 + 2 * K, HW], F32,
                                   name="outt")

                ps = {}
                for dx in range(K):
                    for dy0, ndy in dyq:
                        # product for offsets (dy0..dy0+ndy-1, dx)
                        if lowp and (dx % 2 == 1):
                            src, xoff = x2ot, dx - 1
                        else:
                            src, xoff = x2et, dx
                        in1 = _freeview(
                            src[:, :],
                            [[WPE, ndy], [WPE, H], [1, W]],
                            dy0 * WPE + xoff)
                        in0 = _freeview(
                            x1t[:, :], [[0, ndy], [W, H], [1, W]], 0)
                        pt = ppool.tile([128, max(n for _, n in dyq) * HW],
                                        mmdt, name="pt")
                        out = _freeview(
                            pt[:, :], [[HW, ndy], [W, H], [1, W]], 0)
                        nc.vector.tensor_tensor(out, in0, in1,
                                                op=mybir.AluOpType.mult)

                        for h in range(ndy):
                            dy = dy0 + h
                            o = dy * K + dx
                            g, i, ngrp = grp(dy, dx)
                            first = (i == 0)
                            last = (i == ngrp - 1)
                            m = 2 * ngrp
                            for ch in range(nchunk):
                                if first:
                                    ps[(g, ch)] = pspool.tile(
                                        [28, chunk], F32,
                                        name=f"ps_{j}_{g}_{ch}", tag="ps")
                                nc.tensor.matmul(
                                    ps[(g, ch)][0:m, :],
                                    wtt[:, 32 * o:32 * o + m],
                                    pt[:, h * HW + ch * chunk:
                                       h * HW + (ch + 1) * chunk],
                                    start=first, stop=last)
                                if last:
                                    nc.scalar.copy(
                                        outt[32 * g:32 * g + m,
                                             ch * chunk:(ch + 1) * chunk],
                                        ps[(g, ch)][0:m, :])

                # outt rows are (delta, dxl, dy)-major per group g at base
                # 28g; DRAM offset dim is o = dy*K + (2g + dxl).
                ov = oap.rearrange("(dy dx) t n -> dy dx t n", dx=K)
                for g in range(NG):
                    npx = min(2, K - 2 * g)
                    ng = K * npx
                    for d in range(2):
                        dst = ov[:, 2 * g:2 * g + npx, t0 + d, :]
                        nc.sync.dma_start(
                            dst.rearrange("dy dx n -> dx dy n"),
                            outt[32 * g + d * ng:32 * g + (d + 1) * ng, :])

    nc.compile()
    return nc


def make_wblk(filter_weight, C=C, T=T, K=K, mode="fp16"):
    """Host-side repack of (C, T, K, K) weights into per-pair lhsT blocks
    [NPAIR, 128, 32*K*K]. Offset o = dy*K+dx (group g=dx//2, slot
    i=(dx-2g)*K+dy, group size ng) gets a [128, 2*ng] block at cols
    32o..: col i rows 0..C-1 hold w[:, 2j, o]/C (frame t0), col ng+i
    rows C..2C-1 hold w[:, 2j+1, o]/C (frame t1). Each block is the
    stationary operand of one M=2*ng matmul; PSUM rows are (delta, i)
    delta-major."""
    NO = K * K
    NPAIR = T // 2
    fw = np.asarray(filter_weight, np.float32).reshape(C, T, NO) / C
    wblk = np.zeros((NPAIR, 128, 32 * NO), np.float32)
    for dy in range(K):
        for dx in range(K):
            o = dy * K + dx
            g = dx // 2
            ng = K * min(2, K - 2 * g)
            i = (dx - 2 * g) * K + dy
            wblk[:, 0:C, 32 * o + i] = fw[:, 0::2, o].T
            wblk[:, C:2 * C, 32 * o + ng + i] = fw[:, 1::2, o].T
    if mode == "fp16":
        wblk = wblk.astype(np.float16)
    return wblk


_NC_CACHE = {}


def _get_nc(mode="fp16"):
    if mode not in _NC_CACHE:
        _NC_CACHE[mode] = build(mode=mode)
    return _NC_CACHE[mode]


def kernel(x, filter_weight, mode="fp16"):
    x = np.ascontiguousarray(np.asarray(x, np.float32))
    wblk = make_wblk(filter_weight, mode=mode)
    nc = _get_nc(mode)
    in_maps = [{"x": x[b], "wblk": wblk} for b in range(B)]
    last_err = None
    for _attempt in range(3):
        try:
            res = bass_utils.run_bass_kernel_spmd(
                nc, in_maps, core_ids=list(range(N_CORES)))
            break
        except Exception as e:  # transient device wedge: retry
            last_err = e
    else:
        raise last_err
    out = np.stack([res.results[b]["out"] for b in range(B)])
    return out.reshape(B, NOFF, T, H, W)
